# revision 19
# baseline (speedup 1.0000x reference)
"""DiGCNNet forward on 8 Trainium2 NeuronCores, data-parallel over batch.

Math (per batch b):
  adj = mean_t graph_sigs[b]                  # [30, 30]
  xw  = real[b] @ W                           # [30, 256]
  agg = adj^T @ xw + conv_bias                # [30, 256]
  h   = relu(agg)
  ns  = h @ pool_w + pool_b                   # [30]
  lg  = ns @ head_w^T + head_b                # [7]
  out = softmax(lg)

v2 design (vs baseline): all matmul inputs are host-cast to bf16
(rel err ~3e-3, tolerance 2e-2), halving HBM traffic and running the
PE at 1 cycle/row. 64 batches/core processed as 4 gs loads x 16
batches, groups of 4 batches, pairs of groups sharing 512-wide PSUM.

Per core:
  - gs loads [128=(b16 th8), (tl8, 900)] bf16, host pre-laid so each
    partition is one contiguous 14.4KB run.
  - T-reduce: ones16 [128,16] stationary, 16 matmuls per load into
    PSUM [16,512]+[16,388] (bf16 1 cyc/row).
  - adjs: PSUM -> SBUF bf16 [16, 960] (cols 900:960 zeroed pad).
  - fold: one SBUF->SBUF DMA per group [4,960] -> vfold [128,30]
    (batch k at partitions 32k..32k+30; gpsimd SWDGE, off the shared
    HWDGE path).
  - xw: rt chunks [128,128] stationary (host-padded to 32-aligned
    128-col group blocks), W moving -> xwp [128, 2, 256] per pair.
  - agg: bias matmul (ones1 x cb2) then 8 [30,30] stationary matmuls
    at 32-aligned tile positions, PSUM [128, 512] per pair.
  - relu ACT -> h bf16; pool via one tensor_tensor_reduce per group
    (mult+add) -> ns_all [128, 16] f32.
  - head: one [128,28] matmul over all 16 group-columns; softmax tail
    in f32 (exp with folded bias, 7-block sums via tiny matmuls).
"""

from contextlib import ExitStack

import numpy as np
import ml_dtypes

import concourse.bacc as bacc
import concourse.bass as bass
import concourse.tile as tile
from concourse import mybir
from concourse.bass_utils import run_bass_kernel_spmd

F32 = mybir.dt.float32
BF16 = mybir.dt.bfloat16
FP8 = mybir.dt.float8e4

B, T, N = 512, 64, 30
F_IN, D, C = 512, 256, 7
NCORES = 8
BL = B // NCORES        # 64 batches per core
NLOADS = 4              # gs loads per core
BPL = 16                # batches per load
THL = 8                 # t-high slices (partition dim per batch)
TLO = 8                 # t-low slices (free dim)
GPB = 4                 # batches per group
NG = BL // GPB          # 16 groups
NPAIR = NG // 2         # 8 pairs of groups
NN = N * N              # 900
NNP = 960               # padded to 32*30 for the fold
GCOL = 128              # padded per-group node-column block (4 x 32)


def _build_nc():
    nc = bacc.Bacc(None, target_bir_lowering=False)

    gsl = nc.dram_tensor("gsl", (NLOADS, 128, TLO * NN), mybir.dt.uint8, kind="ExternalInput")
    rtp = nc.dram_tensor("rtp", (128, 4, NG * GCOL), BF16, kind="ExternalInput")
    wtl = nc.dram_tensor("wtl", (128, 4, D), BF16, kind="ExternalInput")
    ones16 = nc.dram_tensor("ones16", (128, BPL), mybir.dt.uint8, kind="ExternalInput")
    ones1 = nc.dram_tensor("ones1", (1, 128), BF16, kind="ExternalInput")
    cb2 = nc.dram_tensor("cb2", (1, 2 * D), BF16, kind="ExternalInput")
    pwb = nc.dram_tensor("pwb", (128, 2, D), BF16, kind="ExternalInput")
    hwblk = nc.dram_tensor("hwblk", (128, GPB * C), F32, kind="ExternalInput")
    hbb = nc.dram_tensor("hbb", (GPB * C, 1), F32, kind="ExternalInput")
    b7 = nc.dram_tensor("b7", (GPB * C, GPB), F32, kind="ExternalInput")
    b7t = nc.dram_tensor("b7t", (GPB, GPB * C), F32, kind="ExternalInput")
    out = nc.dram_tensor("out", (BL, C), F32, kind="ExternalOutput")

    with tile.TileContext(nc) as tc, ExitStack() as ctx:
        consts = ctx.enter_context(tc.tile_pool(name="consts", bufs=1))
        gt_pool = ctx.enter_context(tc.tile_pool(name="gt", bufs=4))
        adjs_pool = ctx.enter_context(tc.tile_pool(name="adjs", bufs=2))
        xwb_pool = ctx.enter_context(tc.tile_pool(name="xwb", bufs=2))
        h_pool = ctx.enter_context(tc.tile_pool(name="h", bufs=2))
        scr_pool = ctx.enter_context(tc.tile_pool(name="scr", bufs=2))
        tail_pool = ctx.enter_context(tc.tile_pool(name="tail", bufs=1))
        tredA_pool = ctx.enter_context(
            tc.tile_pool(name="tredA", bufs=1, space=bass.MemorySpace.PSUM)
        )
        xwp_pool = ctx.enter_context(
            tc.tile_pool(name="xwp", bufs=2, space=bass.MemorySpace.PSUM)
        )
        aggp_pool = ctx.enter_context(
            tc.tile_pool(name="aggp", bufs=2, space=bass.MemorySpace.PSUM)
        )
        smallp_pool = ctx.enter_context(
            tc.tile_pool(name="smallp", bufs=1, space=bass.MemorySpace.PSUM)
        )

        # ---- inputs: gs loads trigger first (sync engine) so the PE's
        # critical stream starts immediately; small constants trigger from
        # the scalar engine to keep the sync queue short.
        gt_tiles = []
        gt_t = gt_pool.tile([128, TLO, NN], FP8, tag="gt")
        nc.sync.dma_start(gt_t[:], gsl[0].bitcast(FP8))
        gt_tiles.append(gt_t)

        ones16_sb = consts.tile([128, BPL], FP8, tag="ones16")
        nc.scalar.dma_start(ones16_sb[:], ones16[:].bitcast(FP8))

        for L in range(1, NLOADS):
            gt_t = gt_pool.tile([128, TLO, NN], FP8, tag="gt")
            eng = nc.scalar if L % 2 else nc.sync
            eng.dma_start(gt_t[:], gsl[L].bitcast(FP8))
            gt_tiles.append(gt_t)

        rt_all = consts.tile([128, 4, NG * GCOL], BF16, tag="rt_all")
        nc.sync.dma_start(rt_all[:, 0:2, :], rtp[:, 0:2, :])
        nc.scalar.dma_start(rt_all[:, 2:4, :], rtp[:, 2:4, :])
        wt_sb = consts.tile([128, 4, D], BF16, tag="wt")
        nc.scalar.dma_start(wt_sb[:], wtl[:])

        def load_const(dram, shape, dtype):
            t = consts.tile(shape, dtype, tag=dram.name)
            nc.scalar.dma_start(t[:], dram[:])
            return t

        ones1_sb = load_const(ones1, [1, 128], BF16)
        cb2_sb = load_const(cb2, [1, 2 * D], BF16)
        pwb_sb = load_const(pwb, [128, 2, D], BF16)
        hw_sb = load_const(hwblk, [128, GPB * C], F32)
        hbb_sb = load_const(hbb, [GPB * C, 1], F32)
        b7_sb = load_const(b7, [GPB * C, GPB], F32)
        b7t_sb = load_const(b7t, [GPB, GPB * C], F32)

        vfold = []
        for g in range(NG):
            vf_t = consts.tile([128, N], BF16, tag=f"vfold{g}")
            vfold.append(vf_t)

        ns_all = consts.tile([128, NG], F32, tag="ns_all")
        pT_all = tredA_pool.tile([128, 1024], F32, tag="T")
        adjs_all = consts.tile([128, NNP], BF16, tag="adjs_all")
        nc.vector.memset(adjs_all[:, NN:NNP].bitcast(F32), 0.0)

        # ---- main loop, software-pipelined: at iteration L the PE does
        # the T-reduce of load L, the xw matmuls of load L's two pairs,
        # and the agg/relu/pool of load L-1's two pairs, so the fold DMAs
        # and PSUM->SBUF copies of a load have a full iteration of slack.
        xwb_tiles = {}

        def emit_tred(L):
            gt_t = gt_tiles[L]
            r0 = 32 * L
            pT = pT_all[r0 : r0 + BPL, :]
            del pT
            for tl in range(TLO):
                nc.tensor.matmul(
                    pT_all[r0 : r0 + BPL, 0:512], ones16_sb[:],
                    gt_t[:, tl, 0:512],
                    start=(tl == 0), stop=(tl == TLO - 1),
                    tile_position=(0, r0), skip_group_check=True,
                )
                nc.tensor.matmul(
                    pT_all[r0 : r0 + BPL, 512:NN], ones16_sb[:],
                    gt_t[:, tl, 512:NN],
                    start=(tl == 0), stop=(tl == TLO - 1),
                    tile_position=(0, r0), skip_group_check=True,
                )
            adjs_t = adjs_all[r0 : r0 + BPL, :]
            if L % 2 == 0:
                nc.scalar.activation(
                    adjs_t[:, 0:NN], pT_all[r0 : r0 + BPL, 0:NN],
                    mybir.ActivationFunctionType.Copy,
                )
            else:
                nc.vector.tensor_copy(
                    adjs_t[:, 0:NN], pT_all[r0 : r0 + BPL, 0:NN]
                )
            if L == NLOADS - 1:
                fold_engines = [nc.sync, nc.scalar, nc.gpsimd, nc.gpsimd]
            else:
                fold_engines = [nc.gpsimd] * GPB
            for m in range(GPB):
                g = GPB * L + m
                fold_engines[m].dma_start(
                    vfold[g][:], adjs_t[GPB * m : GPB * m + GPB, :]
                )

        def emit_xw(p):
            g0 = 2 * p
            xwp = xwp_pool.tile([128, 2, D], F32, tag="xwp")
            for g2 in range(2):
                c0 = (g0 + g2) * GCOL
                for c4 in range(4):
                    nc.tensor.matmul(
                        xwp[:, g2, :],
                        rt_all[:, c4, c0 : c0 + GCOL],
                        wt_sb[:, c4, :],
                        start=(c4 == 0), stop=(c4 == 3),
                    )
            xwb = xwb_pool.tile([128, 2, D], BF16, tag="xwb")
            nc.vector.tensor_copy(xwb[:], xwp[:])
            xwb_tiles[p] = xwb

        def emit_agg(p):
            g0 = 2 * p
            xwb = xwb_tiles.pop(p)
            aggp = aggp_pool.tile([128, 2, D], F32, tag="aggp")
            nc.tensor.matmul(
                aggp[:], ones1_sb[:], cb2_sb[:], start=True, stop=False,
                skip_group_check=True,
            )
            for g2 in range(2):
                for k in range(GPB):
                    p0 = 32 * k
                    nc.tensor.matmul(
                        aggp[p0 : p0 + N, g2, :],
                        vfold[g0 + g2][p0 : p0 + N, :],
                        xwb[p0 : p0 + N, g2, :],
                        start=False, stop=True,
                        tile_position=(p0, p0),
                        skip_group_check=True,
                    )
            h_t = h_pool.tile([128, 2, D], BF16, tag="h")
            nc.scalar.activation(
                h_t[:], aggp[:], mybir.ActivationFunctionType.Relu
            )
            scr = scr_pool.tile([128, 2, D], BF16, tag="scr")
            nc.vector.tensor_mul(scr[:], h_t[:], pwb_sb[:])
            nc.vector.reduce_sum(
                ns_all[:, g0 : g0 + 2], scr[:], axis=mybir.AxisListType.X
            )

        sp = smallp_pool.tile([GPB * C, 3 * NG], F32, tag="sp")

        def emit_head(Lp):
            c0 = GPB * Lp
            nc.tensor.matmul(
                sp[:, c0 : c0 + GPB], hw_sb[:], ns_all[:, c0 : c0 + GPB],
                start=True, stop=True,
            )

        emit_tred(0)
        emit_tred(1)
        emit_xw(0)
        emit_tred(2)
        emit_xw(1)
        emit_agg(0)
        emit_tred(3)
        for p in range(2, NPAIR):
            emit_xw(p)
            emit_agg(p - 1)
            if p % 2 == 0:
                emit_head((p - 1) // 2)
        emit_agg(NPAIR - 1)
        emit_head(NLOADS - 2)
        emit_head(NLOADS - 1)

        # ---- softmax over the 7 classes (f32 tail)
        e_t = tail_pool.tile([GPB * C, NG], F32, tag="e")
        nc.scalar.activation(
            e_t[:], sp[:, 0:NG], mybir.ActivationFunctionType.Exp, bias=hbb_sb[:]
        )
        nc.tensor.matmul(
            sp[0:GPB, NG : 2 * NG], b7_sb[:], e_t[:], start=True, stop=True
        )
        ssb_t = tail_pool.tile([GPB, NG], F32, tag="ssb")
        nc.vector.reciprocal(ssb_t[:], sp[0:GPB, NG : 2 * NG])
        nc.tensor.matmul(
            sp[:, 2 * NG : 3 * NG], b7t_sb[:], ssb_t[:], start=True, stop=True
        )
        res_t = tail_pool.tile([GPB * C, NG], F32, tag="res")
        nc.vector.tensor_mul(res_t[:], e_t[:], sp[:, 2 * NG : 3 * NG])
        nc.sync.dma_start(out.rearrange("(g bi) c -> (bi c) g", bi=GPB), res_t[:])

    nc.compile()
    return nc


_NC_CACHE = None


def _get_nc():
    global _NC_CACHE
    if _NC_CACHE is None:
        _NC_CACHE = _build_nc()
    return _NC_CACHE


def _bf16(x):
    return np.ascontiguousarray(np.asarray(x, dtype=np.float32)).astype(
        ml_dtypes.bfloat16
    )


def _f32c(x):
    return np.ascontiguousarray(np.asarray(x, dtype=np.float32))


def _prepare_in_maps(real, graph_sigs, W, conv_bias, pool_w, pool_b, head_w, head_b):
    real = _f32c(real)
    gs_bf = np.ascontiguousarray(np.asarray(graph_sigs, dtype=np.float32)).astype(ml_dtypes.float8_e4m3fn).view(np.uint8).reshape(B, T, NN)
    W = _f32c(W)

    # W chunked [128(f%...), 4(f//128), 256]: f = c*128 + p
    wt = _bf16(W.reshape(4, 128, D).transpose(1, 0, 2))
    cb2 = np.tile(_bf16(conv_bias).reshape(1, D), (1, 2))
    # pool_w broadcast to 32-aligned batch blocks; pad rows zero
    pwb = np.zeros((128, 2, D), dtype=ml_dtypes.bfloat16)
    pool_w_bf = _bf16(pool_w)
    for k in range(GPB):
        pwb[32 * k : 32 * k + N, 0] = pool_w_bf
        pwb[32 * k : 32 * k + N, 1] = pool_w_bf
    hw_t = _f32c(head_w).T  # [30, 7]
    hwblk = np.zeros((128, GPB * C), dtype=np.float32)
    for k in range(GPB):
        hwblk[32 * k : 32 * k + N, k * C : (k + 1) * C] = hw_t
    # pool_b shifts every node score by a constant; fold into head bias:
    # logits[c] += pool_b * sum_j head_w[c, j]
    hb_eff = _f32c(head_b) + np.float32(np.asarray(pool_b)) * _f32c(head_w).sum(axis=1)
    hbb = np.tile(hb_eff, GPB).reshape(GPB * C, 1)
    ones16 = np.zeros((128, BPL), dtype=ml_dtypes.float8_e4m3fn)
    inv_t = ml_dtypes.float8_e4m3fn(1.0 / T)
    for b in range(BPL):
        ones16[THL * b : THL * (b + 1), b] = inv_t
    ones1 = np.ones((1, 128), dtype=ml_dtypes.bfloat16)
    b7 = np.zeros((GPB * C, GPB), dtype=np.float32)
    for k in range(GPB):
        b7[k * C : (k + 1) * C, k] = 1.0
    b7t = np.ascontiguousarray(b7.T)

    consts = {
        "wtl": wt, "cb2": cb2, "pwb": pwb, "hwblk": hwblk, "hbb": hbb,
        "ones16": ones16.view(np.uint8), "ones1": ones1, "b7": b7, "b7t": b7t,
    }

    in_maps = []
    for core in range(NCORES):
        s = slice(core * BL, (core + 1) * BL)
        # gs: [BL, T, 900] -> [NLOADS, 16b, 8th, 8tl, 900] -> [NLOADS, 128, 7200]
        gsc = gs_bf[s].reshape(NLOADS, BPL, THL, TLO, NN)
        gsl = np.ascontiguousarray(gsc).reshape(NLOADS, 128, TLO * NN)
        # real^T padded to 32-aligned group blocks: [512, 16*128]
        rt = real[s].transpose(2, 0, 1).reshape(F_IN, BL, N)  # [512, 64, 30]
        rtp = np.zeros((F_IN, NG, GPB, 32), dtype=np.float32)
        rtp[:, :, :, 0:N] = rt.reshape(F_IN, NG, GPB, N)
        rtp = rtp.reshape(4, 128, NG * GCOL)  # f = c*128 + p
        rtp = np.ascontiguousarray(rtp.transpose(1, 0, 2)).astype(ml_dtypes.bfloat16)
        in_maps.append({"gsl": gsl, "rtp": rtp, **consts})
    return in_maps


def kernel(real, imag, graph_sigs, W, conv_bias, pool_w, pool_b, head_w, head_b):
    del imag  # unused by the forward pass
    in_maps = _prepare_in_maps(
        real, graph_sigs, W, conv_bias, pool_w, pool_b, head_w, head_b
    )
    nc = _get_nc()
    res = run_bass_kernel_spmd(nc, in_maps, core_ids=list(range(NCORES)))
    return np.concatenate([res.results[c]["out"] for c in range(NCORES)], axis=0)
